# revision 28
# baseline (speedup 1.0000x reference)
"""Trainium2 Bass kernel for nn_AttentionHead_Hybrid2 (B=16, S=2048, D=64).

Reference, per batch b:
    V = x @ Wv;  q = x @ Wq;  k = x @ Wk          (q, k scalar per token)
    A[i,j] = -(q_i - k_j)^2 / 8
    out = softmax_j(A) @ V

Softmax over j is shift-invariant, so the -q_i^2 term drops and
    P[i,j] ∝ exp(q_i k_j / 4) * w_j,   w_j = exp(-k_j^2 / 8).
With q' = q/4, k' = k/4 and the Taylor series of exp, the whole attention
collapses to rank-NTERMS linear algebra (NTERMS=12 reaches ~8e-4):
    f_n(i) = q'_i^n                  (features of queries)
    g_n(j) = k'_j^n * w_j            (features of keys)
    M[c,n] = sum_j [x|1](j,c) g_n(j)     ("G-matmul", PE, fp16)
    A[n,:] = (4^n/n!) * [M[:64,n]^T Wv | M[64,n]]    (fixup matmul, tf32)
    out[i] = (sum_n f_n(i) A[n,:64]) / (sum_n f_n(i) A[n,64])
V is never materialized and x is never transposed: the only PE work is
16 accumulating G-matmuls + 16 finals per batch plus one tiny fixup.
q,k are computed on the DVE (fp16 multiply + pairwise-fold tree + f32
reduce; the 2-byte dtype doubles DVE throughput), features are chained in
fp16 (the 1/4 rescale keeps q'^n, k'^n w in fp16 range; the 4^n/n! Taylor
coefficients are folded into A), and F^T is produced by the DMA-engine
XBAR transpose (out[P,E,I] = ff[I, 128E+P], with f staged into the low 16
columns of each 128-wide block; source columns >= the 16 output partitions
are dropped).  Batch 0's feature chain runs on the DVE and batch 1's on
GPSIMD so the two batches' prep overlaps.  Input DMAs are issued batch-0
first on one HWDGE queue so batch 0's data outruns batch 1's; output DMAs
alternate between the sync and scalar queues.  End-to-end error vs the
f64 reference is ~8e-4 (fp16 rounding), well inside the 2e-2 gate.

Sharding: data-parallel over batch - 2 batches per core on 8 NeuronCores,
no collectives.
"""
import math

import numpy as np

import concourse.tile as tile
from concourse import bacc, mybir
from concourse.bass_utils import run_bass_kernel_spmd

B, S, D = 16, 2048, 64
NCORES = 8
BPC = B // NCORES  # batches per core
NT = S // 128  # 128-token tiles per batch
NTERMS = 12
F32 = mybir.dt.float32
F32R = mybir.dt.float32r
F16 = mybir.dt.float16
AF = mybir.ActivationFunctionType
AX = mybir.AxisListType
OP = mybir.AluOpType


def build_nc():
    nc = bacc.Bacc(None, target_bir_lowering=False)
    xin = nc.declare_dram_parameter("xin", [BPC, S, 66], F16, isOutput=False)
    wqk = nc.declare_dram_parameter("wqk", [128, 2 * D], F16, isOutput=False)
    w66 = nc.declare_dram_parameter("w66", [66, 66], F32R, isOutput=False)
    coef = nc.declare_dram_parameter("coef", [32, 1], F32, isOutput=False)
    out = nc.declare_dram_parameter("out", [BPC, S, D], F32, isOutput=True)

    with tile.TileContext(nc) as tc:
        with (
            tc.tile_pool(name="const", bufs=1) as constp,
            tc.tile_pool(name="xob", bufs=2) as xobp,
            tc.tile_pool(name="tmp", bufs=1) as tmpp,
            tc.tile_pool(name="small", bufs=2) as smallp,
            tc.tile_pool(name="fg", bufs=2) as fgp,
            tc.tile_pool(name="ft", bufs=2) as ftp,
            tc.tile_pool(name="ost", bufs=8) as ostp,
            tc.tile_pool(name="ps_m", bufs=2, space="PSUM") as ps_m,
            tc.tile_pool(name="ps_a", bufs=2, space="PSUM") as ps_a,
            tc.tile_pool(name="ps_po", bufs=4, space="PSUM") as ps_po,
        ):
            wqk_sb = constp.tile([128, 2 * D], F16)
            nc.sync.dma_start(wqk_sb[:], wqk[:])
            w66_sb = constp.tile([66, 66], F32R)
            coef_sb = constp.tile([32, 1], F32)

            # input arrives host-converted to fp16 with the ones columns
            # pre-appended ([x|1|1], 66 wide) — half the HBM traffic and no
            # on-device convert.  Batch 0 first on the scalar HWDGE.
            xobs = []
            for b in range(BPC):
                xob = xobp.tile([128, NT * 66], F16, tag="xob")
                xobs.append(xob)
            xv0 = xin[0].rearrange("(p a) d -> p a d", a=NT)
            xob0v = xobs[0][:].rearrange("p (a d) -> p a d", d=66)
            xv1 = xin[1].rearrange("(p a) d -> p a d", a=NT)
            xob1v = xobs[1][:].rearrange("p (a d) -> p a d", d=66)
            nc.scalar.dma_start(xob0v[:, 0:8, :], xv0[:, 0:8, :])
            nc.scalar.dma_start(xob0v[:, 8:16, :], xv0[:, 8:16, :])
            nc.scalar.dma_start(xob1v[:, 0:8, :], xv1[:, 0:8, :])
            nc.scalar.dma_start(xob1v[:, 8:16, :], xv1[:, 8:16, :])
            nc.sync.dma_start(w66_sb[:], w66[:])
            nc.sync.dma_start(coef_sb[:], coef[:])

            # PE warm-up: release the HAM clock gate while input DMAs fly.
            junk = smallp.tile([128, 256], F16, tag="junk")
            nc.gpsimd.memset(junk[:], 0.0)
            for _ in range(8):
                pjw = ps_po.tile([128, 264], F32, tag="po")
                nc.tensor.matmul(
                    pjw[:, 0:256], junk[:, 0:128], junk[:], start=True, stop=True
                )

            qks, fgs = [], []
            wqkv = wqk_sb[:].rearrange("p (o e d) -> p o e d", o=1, e=2)

            def prep(b):
                """q/k dot products + feature chain for batch b."""
                xobv = xobs[b][:].rearrange("p (a d) -> p a d", d=66)

                tA = tmpp.tile([128, NT * 2 * 64], F16, tag="tA")
                tAv = tA[:].rearrange("p (t e d) -> p t e d", t=NT, e=2)
                for h in range(2):
                    sl = slice(8 * h, 8 * h + 8)
                    xh = xobv[:, sl, 0:64].rearrange(
                        "p t (o d) -> p t o d", o=1
                    ).broadcast_to([128, 8, 2, D])
                    wh = wqkv.broadcast_to([128, 8, 2, D])
                    nc.vector.tensor_mul(tAv[:, sl, :, :], xh, wh)
                tB = tmpp.tile([128, NT * 2 * 32], F16, tag="tB")
                tBv = tB[:].rearrange("p (t e d) -> p t e d", t=NT, e=2)
                nc.vector.tensor_add(tBv, tAv[:, :, :, 0:32], tAv[:, :, :, 32:64])
                tC = tmpp.tile([128, NT * 2 * 16], F16, tag="tC")
                tCv = tC[:].rearrange("p (t e d) -> p t e d", t=NT, e=2)
                nc.vector.tensor_add(tCv, tBv[:, :, :, 0:16], tBv[:, :, :, 16:32])
                tD = tmpp.tile([128, NT * 2 * 8], F16, tag="tD")
                tDv = tD[:].rearrange("p (t e d) -> p t e d", t=NT, e=2)
                nc.vector.tensor_add(tDv, tCv[:, :, :, 0:8], tCv[:, :, :, 8:16])
                # fp16 q', k' straight out of the fold tree (4 summands of
                # bounded magnitude: fp16 rounding is input-perturbation-safe)
                qkh = smallp.tile([128, NT * 2], F16, tag="qkh")
                qkhv = qkh[:].rearrange("p (t e) -> p t e", e=2)
                with nc.allow_low_precision("fp16 qk fold output"):
                    nc.vector.tensor_reduce(qkhv, tDv, AX.X, OP.add)
                # feature chain in fp16: col (t, e, n): e=0 -> q'^n, e=1 -> k'^n w
                veng = nc.vector
                fg = fgp.tile([128, NT * 2 * 32], F16, tag="fg")
                fgs.append(fg)
                fgv = fg[:].rearrange("p (t e n) -> p t e n", e=2, n=32)
                nc.gpsimd.memset(fgv[:, :, 0:1, 0:1], 1.0)
                sqf = smallp.tile([128, NT], F32, tag="sqf")
                nc.scalar.activation(
                    sqf[:].rearrange("p (t o) -> p t o", o=1),
                    qkhv[:, :, 1:2],
                    AF.Square,
                )
                # w = exp(-2 k'^2) written directly into the g_0 feature slot
                nc.scalar.activation(
                    fgv[:, :, 1:2, 0:1],
                    sqf[:].rearrange("p (t e n) -> p t e n", e=1, n=1),
                    AF.Exp,
                    scale=-2.0,
                )
                qkh4 = qkhv.rearrange("p t (e o) -> p t e o", o=1)
                veng.tensor_mul(fgv[:, :, :, 1:2], fgv[:, :, :, 0:1], qkh4)
                p2 = smallp.tile([128, NT * 2], F16, tag="p2")
                p2v = p2[:].rearrange("p (t e) -> p t e", e=2)
                veng.tensor_mul(p2v, qkhv, qkhv)
                p4 = smallp.tile([128, NT * 2], F16, tag="p4")
                p4v = p4[:].rearrange("p (t e) -> p t e", e=2)
                veng.tensor_mul(p4v, p2v, p2v)
                veng.tensor_mul(
                    fgv[:, :, :, 2:4],
                    fgv[:, :, :, 0:2],
                    p2v.rearrange("p t (e o) -> p t e o", o=1).broadcast_to(
                        [128, NT, 2, 2]
                    ),
                )
                p4r = smallp.tile([128, NT * 2 * 4], F16, tag="p4r")
                p4rv = p4r[:].rearrange("p (t e n) -> p t e n", e=2, n=4)
                veng.tensor_copy(
                    p4rv,
                    p4v.rearrange("p t (e o) -> p t e o", o=1).broadcast_to(
                        [128, NT, 2, 4]
                    ),
                )
                veng.tensor_mul(fgv[:, :, :, 4:8], fgv[:, :, :, 0:4], p4rv)
                veng.tensor_mul(fgv[:, :, :, 8:12], fgv[:, :, :, 4:8], p4rv)

            for b in range(BPC):
                prep(b)

            # per batch: G-matmuls + A fixup + coef scale + F^T + finals
            for b in range(BPC):
                xobv = xobs[b][:].rearrange("p (a d) -> p a d", d=66)
                fgv = fgs[b][:].rearrange("p (t e n) -> p t e n", e=2, n=32)
                pm = ps_m.tile([66, 32], F32, tag="m")
                for t in range(NT):
                    nc.tensor.matmul(
                        pm[:],
                        xobv[:, t, :],
                        fgv[:, t, 1, :],
                        start=(t == 0),
                        stop=(t == NT - 1),
                    )
                msb = smallp.tile([66, 32], F32R, tag="msb")
                nc.scalar.copy(msb[:], pm[:])
                pa = ps_a.tile([32, 66], F32, tag="a")
                nc.tensor.matmul(pa[:], msb[:], w66_sb[:], start=True, stop=True)
                atb = smallp.tile([32, 66], F16, tag="atb")
                nc.vector.tensor_scalar_mul(
                    atb[0:NTERMS, :], pa[0:NTERMS, :], coef_sb[0:NTERMS, :]
                )
                nc.gpsimd.memset(fgv[:, :, 0:1, 12:16], 0.0)
                if b == 0:
                    # batch 0: DVE stream-transpose (DVE frees up right after
                    # the chains; keeps the sync DGE clear for batch 1's XBAR)
                    ftb = ftp.tile([32, S], F16, tag="ftb32")
                    ftv = ftb[:].rearrange("p (t k i) -> p t k i", t=NT, k=4)
                    for kb in range(4):
                        nc.vector.transpose(
                            ftv[:, :, kb : kb + 1, :],
                            fgv[32 * kb : 32 * kb + 32, :, 0:1, :],
                        )
                else:
                    # batch 1: XBAR transpose — f staged into the low 16
                    # columns of 128-wide blocks; out[P,E,I] = ff[I, 128E+P],
                    # source columns >= the 16 out partitions are dropped.
                    ff = fgp.tile([128, 128 * NT], F16, tag="ff")
                    ffv = ff[:].rearrange("p (t c) -> p t c", c=128)
                    nc.vector.tensor_copy(ffv[:, :, 0:16], fgv[:, :, 0, 0:16])
                    ftb = ftp.tile([16, S], F16, tag="ftb")
                    nc.sync.dma_start_transpose(
                        ftb[:].rearrange("p (t i) -> p t i", i=128), ff[:]
                    )

                # finals + normalize + output DMA for this batch
                ov = out[b].rearrange("(p a) d -> p a d", a=NT)
                for g in range(4):
                    po = ps_po.tile([128, 264], F32, tag="po")
                    pov = po[:].rearrange("p (k e) -> p k e", e=66)
                    for t2 in range(4):
                        t = 4 * g + t2
                        nc.tensor.matmul(
                            po[:, 66 * t2 : 66 * (t2 + 1)],
                            ftb[0:NTERMS, 128 * t : 128 * (t + 1)],
                            atb[0:NTERMS, :],
                            start=True,
                            stop=True,
                        )
                    rec = smallp.tile([128, 4], F32, tag="rec")
                    nc.vector.reciprocal(
                        rec[:].rearrange("p (k o) -> p k o", o=1), pov[:, :, 64:65]
                    )
                    ost = ostp.tile([128, 4 * 64], F32, tag="ost")
                    ostv = ost[:].rearrange("p (k d) -> p k d", k=4)
                    nc.vector.tensor_mul(
                        ostv,
                        pov[:, :, 0:64],
                        rec[:].rearrange("p (k o) -> p k o", o=1).broadcast_to(
                            [128, 4, 64]
                        ),
                    )
                    oeng = nc.sync if g % 2 == 0 else nc.scalar
                    oeng.dma_start(ov[:, 4 * g : 4 * g + 4, :], ostv)
    nc.compile()
    return nc


_NC_CACHE = None


def _get_nc():
    global _NC_CACHE
    if _NC_CACHE is None:
        _NC_CACHE = build_nc()
    return _NC_CACHE


def make_in_maps(input1, Wv, Wq, Wk):
    input1 = np.asarray(input1, dtype=np.float32)
    xin16 = np.empty((B, S, 66), np.float16)
    xin16[:, :, 0:64] = input1
    xin16[:, :, 64:66] = 1.0
    Wv = np.asarray(Wv, dtype=np.float32)
    Wq = np.asarray(Wq, dtype=np.float32)
    Wk = np.asarray(Wk, dtype=np.float32)
    wqk1 = np.stack([Wq / 4.0, Wk / 4.0]).reshape(1, 2 * D)
    wqk = np.broadcast_to(wqk1, (128, 2 * D)).astype(np.float16)
    w66 = np.zeros((66, 66), np.float32)
    w66[:64, :64] = Wv
    w66[64, 64] = 1.0
    coef = np.zeros((32, 1), np.float32)
    for n in range(NTERMS):
        coef[n] = 4.0**n / float(math.factorial(n))
    return [
        {
            "xin": np.ascontiguousarray(xin16[i * BPC : (i + 1) * BPC]),
            "wqk": np.ascontiguousarray(wqk),
            "w66": w66,
            "coef": coef,
        }
        for i in range(NCORES)
    ]


def kernel(input1, Wv, Wq, Wk):
    nc = _get_nc()
    in_maps = make_in_maps(input1, Wv, Wq, Wk)
    res = run_bass_kernel_spmd(nc, in_maps, core_ids=list(range(NCORES)))
    return np.concatenate([res.results[i]["out"] for i in range(NCORES)], axis=0)


# revision 29
# speedup vs baseline: 1.0063x; 1.0063x over previous
"""Trainium2 Bass kernel for nn_AttentionHead_Hybrid2 (B=16, S=2048, D=64).

Reference, per batch b:
    V = x @ Wv;  q = x @ Wq;  k = x @ Wk          (q, k scalar per token)
    A[i,j] = -(q_i - k_j)^2 / 8
    out = softmax_j(A) @ V

Softmax over j is shift-invariant, so the -q_i^2 term drops and
    P[i,j] ∝ exp(q_i k_j / 4) * w_j,   w_j = exp(-k_j^2 / 8).
With q' = q/4, k' = k/4 and the Taylor series of exp, the whole attention
collapses to rank-NTERMS linear algebra (NTERMS=12 reaches ~8e-4):
    f_n(i) = q'_i^n                  (features of queries)
    g_n(j) = k'_j^n * w_j            (features of keys)
    M[c,n] = sum_j [x|1](j,c) g_n(j)     ("G-matmul", PE, fp16)
    A[n,:] = (4^n/n!) * [M[:64,n]^T Wv | M[64,n]]    (fixup matmul, tf32)
    out[i] = (sum_n f_n(i) A[n,:64]) / (sum_n f_n(i) A[n,64])
V is never materialized and x is never transposed: the only PE work is
16 accumulating G-matmuls + 16 finals per batch plus one tiny fixup.
q,k are computed on the DVE (fp16 multiply + pairwise-fold tree + f32
reduce; the 2-byte dtype doubles DVE throughput), features are chained in
fp16 (the 1/4 rescale keeps q'^n, k'^n w in fp16 range; the 4^n/n! Taylor
coefficients are folded into A), and F^T is produced by the DMA-engine
XBAR transpose (out[P,E,I] = ff[I, 128E+P], with f staged into the low 16
columns of each 128-wide block; source columns >= the 16 output partitions
are dropped).  Batch 0's feature chain runs on the DVE and batch 1's on
GPSIMD so the two batches' prep overlaps.  Input DMAs are issued batch-0
first on one HWDGE queue so batch 0's data outruns batch 1's; output DMAs
alternate between the sync and scalar queues.  End-to-end error vs the
f64 reference is ~8e-4 (fp16 rounding), well inside the 2e-2 gate.

Sharding: data-parallel over batch - 2 batches per core on 8 NeuronCores,
no collectives.
"""
import math

import numpy as np

import concourse.tile as tile
from concourse import bacc, mybir
from concourse.bass_utils import run_bass_kernel_spmd

B, S, D = 16, 2048, 64
NCORES = 8
BPC = B // NCORES  # batches per core
NT = S // 128  # 128-token tiles per batch
NTERMS = 12
F32 = mybir.dt.float32
F32R = mybir.dt.float32r
F16 = mybir.dt.float16
AF = mybir.ActivationFunctionType
AX = mybir.AxisListType
OP = mybir.AluOpType


def build_nc():
    nc = bacc.Bacc(None, target_bir_lowering=False)
    xin = nc.declare_dram_parameter("xin", [BPC, S, 66], F16, isOutput=False)
    wqk = nc.declare_dram_parameter("wqk", [128, 2 * D], F16, isOutput=False)
    w66 = nc.declare_dram_parameter("w66", [66, 66], F32R, isOutput=False)
    coef = nc.declare_dram_parameter("coef", [32, 1], F32, isOutput=False)
    out = nc.declare_dram_parameter("out", [BPC, S, D], F32, isOutput=True)

    with tile.TileContext(nc) as tc:
        with (
            tc.tile_pool(name="const", bufs=1) as constp,
            tc.tile_pool(name="xob", bufs=2) as xobp,
            tc.tile_pool(name="tmp", bufs=1) as tmpp,
            tc.tile_pool(name="small", bufs=2) as smallp,
            tc.tile_pool(name="fg", bufs=2) as fgp,
            tc.tile_pool(name="ft", bufs=2) as ftp,
            tc.tile_pool(name="ost", bufs=8) as ostp,
            tc.tile_pool(name="ps_m", bufs=2, space="PSUM") as ps_m,
            tc.tile_pool(name="ps_a", bufs=2, space="PSUM") as ps_a,
            tc.tile_pool(name="ps_po", bufs=4, space="PSUM") as ps_po,
        ):
            wqk_sb = constp.tile([128, 2 * D], F16)
            nc.sync.dma_start(wqk_sb[:], wqk[:])
            w66_sb = constp.tile([66, 66], F32R)
            coef_sb = constp.tile([32, 1], F32)

            # input arrives host-converted to fp16 with the ones columns
            # pre-appended ([x|1|1], 66 wide) — half the HBM traffic and no
            # on-device convert.  Batch 0 first on the scalar HWDGE.
            xobs = []
            for b in range(BPC):
                xob = xobp.tile([128, NT * 66], F16, tag="xob")
                xobs.append(xob)
            xv0 = xin[0].rearrange("(p a) d -> p a d", a=NT)
            xob0v = xobs[0][:].rearrange("p (a d) -> p a d", d=66)
            xv1 = xin[1].rearrange("(p a) d -> p a d", a=NT)
            xob1v = xobs[1][:].rearrange("p (a d) -> p a d", d=66)
            nc.scalar.dma_start(xob0v[:, 0:8, :], xv0[:, 0:8, :])
            nc.scalar.dma_start(xob0v[:, 8:16, :], xv0[:, 8:16, :])
            nc.scalar.dma_start(xob1v[:, 0:8, :], xv1[:, 0:8, :])
            nc.scalar.dma_start(xob1v[:, 8:16, :], xv1[:, 8:16, :])
            nc.sync.dma_start(w66_sb[:], w66[:])
            nc.sync.dma_start(coef_sb[:], coef[:])

            # PE warm-up: release the HAM clock gate while input DMAs fly.
            junk = smallp.tile([128, 256], F16, tag="junk")
            nc.gpsimd.memset(junk[:], 0.0)
            for _ in range(8):
                pjw = ps_po.tile([128, 264], F32, tag="po")
                nc.tensor.matmul(
                    pjw[:, 0:256], junk[:, 0:128], junk[:], start=True, stop=True
                )

            qks, fgs = [], []
            wqkv = wqk_sb[:].rearrange("p (o e d) -> p o e d", o=1, e=2)

            def prep(b):
                """q/k dot products + feature chain for batch b."""
                xobv = xobs[b][:].rearrange("p (a d) -> p a d", d=66)

                tA = tmpp.tile([128, NT * 2 * 64], F16, tag="tA")
                tAv = tA[:].rearrange("p (t e d) -> p t e d", t=NT, e=2)
                for h in range(2):
                    sl = slice(8 * h, 8 * h + 8)
                    xh = xobv[:, sl, 0:64].rearrange(
                        "p t (o d) -> p t o d", o=1
                    ).broadcast_to([128, 8, 2, D])
                    wh = wqkv.broadcast_to([128, 8, 2, D])
                    nc.vector.tensor_mul(tAv[:, sl, :, :], xh, wh)
                tB = tmpp.tile([128, NT * 2 * 32], F16, tag="tB")
                tBv = tB[:].rearrange("p (t e d) -> p t e d", t=NT, e=2)
                nc.vector.tensor_add(tBv, tAv[:, :, :, 0:32], tAv[:, :, :, 32:64])
                tC = tmpp.tile([128, NT * 2 * 16], F16, tag="tC")
                tCv = tC[:].rearrange("p (t e d) -> p t e d", t=NT, e=2)
                nc.vector.tensor_add(tCv, tBv[:, :, :, 0:16], tBv[:, :, :, 16:32])
                tD = tmpp.tile([128, NT * 2 * 8], F16, tag="tD")
                tDv = tD[:].rearrange("p (t e d) -> p t e d", t=NT, e=2)
                nc.vector.tensor_add(tDv, tCv[:, :, :, 0:8], tCv[:, :, :, 8:16])
                # fp16 q', k' straight out of the fold tree (4 summands of
                # bounded magnitude: fp16 rounding is input-perturbation-safe)
                qkh = smallp.tile([128, NT * 2], F16, tag="qkh")
                qkhv = qkh[:].rearrange("p (t e) -> p t e", e=2)
                with nc.allow_low_precision("fp16 qk fold output"):
                    nc.vector.tensor_reduce(qkhv, tDv, AX.X, OP.add)
                # feature chain in fp16: col (t, e, n): e=0 -> q'^n, e=1 -> k'^n w
                veng = nc.vector
                fg = fgp.tile([128, NT * 2 * 32], F16, tag="fg")
                fgs.append(fg)
                fgv = fg[:].rearrange("p (t e n) -> p t e n", e=2, n=32)
                nc.gpsimd.memset(fgv[:, :, 0:1, 0:1], 1.0)
                sqf = smallp.tile([128, NT], F32, tag="sqf")
                nc.scalar.activation(
                    sqf[:].rearrange("p (t o) -> p t o", o=1),
                    qkhv[:, :, 1:2],
                    AF.Square,
                )
                # w = exp(-2 k'^2) written directly into the g_0 feature slot
                nc.scalar.activation(
                    fgv[:, :, 1:2, 0:1],
                    sqf[:].rearrange("p (t e n) -> p t e n", e=1, n=1),
                    AF.Exp,
                    scale=-2.0,
                )
                qkh4 = qkhv.rearrange("p t (e o) -> p t e o", o=1)
                p2 = smallp.tile([128, NT * 2], F16, tag="p2")
                p2v = p2[:].rearrange("p (t e) -> p t e", e=2)
                veng.tensor_mul(p2v, qkhv, qkhv)
                p4 = smallp.tile([128, NT * 2], F16, tag="p4")
                p4v = p4[:].rearrange("p (t e) -> p t e", e=2)
                veng.tensor_mul(p4v, p2v, p2v)
                p4r = smallp.tile([128, NT * 2 * 4], F16, tag="p4r")
                p4rv = p4r[:].rearrange("p (t e n) -> p t e n", e=2, n=4)
                veng.tensor_copy(
                    p4rv,
                    p4v.rearrange("p t (e o) -> p t e o", o=1).broadcast_to(
                        [128, NT, 2, 4]
                    ),
                )
                p2b = p2v.rearrange("p t (e o) -> p t e o", o=1)
                # f-side chain first: depends only on qkh (no Act round-trip),
                # so the F^T transpose can launch before the g-side finishes
                veng.tensor_copy(fgv[:, :, 0:1, 1:2], qkh4[:, :, 0:1, :])
                veng.tensor_mul(
                    fgv[:, :, 0:1, 2:4],
                    fgv[:, :, 0:1, 0:2],
                    p2b[:, :, 0:1, :].broadcast_to([128, NT, 1, 2]),
                )
                veng.tensor_mul(
                    fgv[:, :, 0:1, 4:8], fgv[:, :, 0:1, 0:4], p4rv[:, :, 0:1, :]
                )
                veng.tensor_mul(
                    fgv[:, :, 0:1, 8:12], fgv[:, :, 0:1, 4:8], p4rv[:, :, 0:1, :]
                )
                # g-side chain (waits for w = exp(-2k'^2) in g_0)
                veng.tensor_mul(fgv[:, :, 1:2, 1:2], fgv[:, :, 1:2, 0:1], qkh4[:, :, 1:2, :])
                veng.tensor_mul(
                    fgv[:, :, 1:2, 2:4],
                    fgv[:, :, 1:2, 0:2],
                    p2b[:, :, 1:2, :].broadcast_to([128, NT, 1, 2]),
                )
                veng.tensor_mul(
                    fgv[:, :, 1:2, 4:8], fgv[:, :, 1:2, 0:4], p4rv[:, :, 1:2, :]
                )
                veng.tensor_mul(
                    fgv[:, :, 1:2, 8:12], fgv[:, :, 1:2, 4:8], p4rv[:, :, 1:2, :]
                )

            for b in range(BPC):
                prep(b)

            # per batch: G-matmuls + A fixup + coef scale + F^T + finals
            for b in range(BPC):
                xobv = xobs[b][:].rearrange("p (a d) -> p a d", d=66)
                fgv = fgs[b][:].rearrange("p (t e n) -> p t e n", e=2, n=32)
                pm = ps_m.tile([66, 32], F32, tag="m")
                for t in range(NT):
                    nc.tensor.matmul(
                        pm[:],
                        xobv[:, t, :],
                        fgv[:, t, 1, :],
                        start=(t == 0),
                        stop=(t == NT - 1),
                    )
                msb = smallp.tile([66, 32], F32R, tag="msb")
                nc.scalar.copy(msb[:], pm[:])
                pa = ps_a.tile([32, 66], F32, tag="a")
                nc.tensor.matmul(pa[:], msb[:], w66_sb[:], start=True, stop=True)
                atb = smallp.tile([32, 66], F16, tag="atb")
                nc.vector.tensor_scalar_mul(
                    atb[0:NTERMS, :], pa[0:NTERMS, :], coef_sb[0:NTERMS, :]
                )
                nc.gpsimd.memset(fgv[:, :, 0:1, 12:16], 0.0)
                if b == 0:
                    # batch 0: DVE stream-transpose (DVE frees up right after
                    # the chains; keeps the sync DGE clear for batch 1's XBAR)
                    ftb = ftp.tile([32, S], F16, tag="ftb32")
                    ftv = ftb[:].rearrange("p (t k i) -> p t k i", t=NT, k=4)
                    for kb in range(4):
                        nc.vector.transpose(
                            ftv[:, :, kb : kb + 1, :],
                            fgv[32 * kb : 32 * kb + 32, :, 0:1, :],
                        )
                else:
                    # batch 1: XBAR transpose — f staged into the low 16
                    # columns of 128-wide blocks; out[P,E,I] = ff[I, 128E+P],
                    # source columns >= the 16 out partitions are dropped.
                    ff = fgp.tile([128, 128 * NT], F16, tag="ff")
                    ffv = ff[:].rearrange("p (t c) -> p t c", c=128)
                    nc.vector.tensor_copy(ffv[:, :, 0:16], fgv[:, :, 0, 0:16])
                    ftb = ftp.tile([16, S], F16, tag="ftb")
                    nc.sync.dma_start_transpose(
                        ftb[:].rearrange("p (t i) -> p t i", i=128), ff[:]
                    )

                # finals + normalize + output DMA for this batch
                ov = out[b].rearrange("(p a) d -> p a d", a=NT)
                for g in range(4):
                    po = ps_po.tile([128, 264], F32, tag="po")
                    pov = po[:].rearrange("p (k e) -> p k e", e=66)
                    for t2 in range(4):
                        t = 4 * g + t2
                        nc.tensor.matmul(
                            po[:, 66 * t2 : 66 * (t2 + 1)],
                            ftb[0:NTERMS, 128 * t : 128 * (t + 1)],
                            atb[0:NTERMS, :],
                            start=True,
                            stop=True,
                        )
                    rec = smallp.tile([128, 4], F32, tag="rec")
                    nc.vector.reciprocal(
                        rec[:].rearrange("p (k o) -> p k o", o=1), pov[:, :, 64:65]
                    )
                    ost = ostp.tile([128, 4 * 64], F32, tag="ost")
                    ostv = ost[:].rearrange("p (k d) -> p k d", k=4)
                    nc.vector.tensor_mul(
                        ostv,
                        pov[:, :, 0:64],
                        rec[:].rearrange("p (k o) -> p k o", o=1).broadcast_to(
                            [128, 4, 64]
                        ),
                    )
                    oeng = nc.sync if g % 2 == 0 else nc.scalar
                    oeng.dma_start(ov[:, 4 * g : 4 * g + 4, :], ostv)
    nc.compile()
    return nc


_NC_CACHE = None


def _get_nc():
    global _NC_CACHE
    if _NC_CACHE is None:
        _NC_CACHE = build_nc()
    return _NC_CACHE


def make_in_maps(input1, Wv, Wq, Wk):
    input1 = np.asarray(input1, dtype=np.float32)
    xin16 = np.empty((B, S, 66), np.float16)
    xin16[:, :, 0:64] = input1
    xin16[:, :, 64:66] = 1.0
    Wv = np.asarray(Wv, dtype=np.float32)
    Wq = np.asarray(Wq, dtype=np.float32)
    Wk = np.asarray(Wk, dtype=np.float32)
    wqk1 = np.stack([Wq / 4.0, Wk / 4.0]).reshape(1, 2 * D)
    wqk = np.broadcast_to(wqk1, (128, 2 * D)).astype(np.float16)
    w66 = np.zeros((66, 66), np.float32)
    w66[:64, :64] = Wv
    w66[64, 64] = 1.0
    coef = np.zeros((32, 1), np.float32)
    for n in range(NTERMS):
        coef[n] = 4.0**n / float(math.factorial(n))
    return [
        {
            "xin": np.ascontiguousarray(xin16[i * BPC : (i + 1) * BPC]),
            "wqk": np.ascontiguousarray(wqk),
            "w66": w66,
            "coef": coef,
        }
        for i in range(NCORES)
    ]


def kernel(input1, Wv, Wq, Wk):
    nc = _get_nc()
    in_maps = make_in_maps(input1, Wv, Wq, Wk)
    res = run_bass_kernel_spmd(nc, in_maps, core_ids=list(range(NCORES)))
    return np.concatenate([res.results[i]["out"] for i in range(NCORES)], axis=0)


# revision 30
# speedup vs baseline: 1.0733x; 1.0666x over previous
"""Trainium2 Bass kernel for nn_AttentionHead_Hybrid2 (B=16, S=2048, D=64).

Reference, per batch b:
    V = x @ Wv;  q = x @ Wq;  k = x @ Wk          (q, k scalar per token)
    A[i,j] = -(q_i - k_j)^2 / 8
    out = softmax_j(A) @ V

Softmax over j is shift-invariant, so the -q_i^2 term drops and
    P[i,j] ∝ exp(q_i k_j / 4) * w_j,   w_j = exp(-k_j^2 / 8).
With q' = q/4, k' = k/4 and the Taylor series of exp, the whole attention
collapses to rank-NTERMS linear algebra (NTERMS=12 reaches ~8e-4):
    f_n(i) = q'_i^n                  (features of queries)
    g_n(j) = k'_j^n * w_j            (features of keys)
    M[c,n] = sum_j [x|1](j,c) g_n(j)     ("G-matmul", PE, fp16)
    A[n,:] = (4^n/n!) * [M[:64,n]^T Wv | M[64,n]]    (fixup matmul, tf32)
    out[i] = (sum_n f_n(i) A[n,:64]) / (sum_n f_n(i) A[n,64])
V is never materialized and x is never transposed: the only PE work is
16 accumulating G-matmuls + 16 finals per batch plus one tiny fixup.
q,k are computed on the DVE (fp16 multiply + pairwise-fold tree + f32
reduce; the 2-byte dtype doubles DVE throughput), features are chained in
fp16 (the 1/4 rescale keeps q'^n, k'^n w in fp16 range; the 4^n/n! Taylor
coefficients are folded into A), and F^T is produced by the DMA-engine
XBAR transpose (out[P,E,I] = ff[I, 128E+P], with f staged into the low 16
columns of each 128-wide block; source columns >= the 16 output partitions
are dropped).  Batch 0's feature chain runs on the DVE and batch 1's on
GPSIMD so the two batches' prep overlaps.  Input DMAs are issued batch-0
first on one HWDGE queue so batch 0's data outruns batch 1's; output DMAs
alternate between the sync and scalar queues.  End-to-end error vs the
f64 reference is ~8e-4 (fp16 rounding), well inside the 2e-2 gate.

Sharding: data-parallel over batch - 2 batches per core on 8 NeuronCores,
no collectives.
"""
import math

import numpy as np

import concourse.tile as tile
from concourse import bacc, mybir
from concourse.bass_utils import run_bass_kernel_spmd

B, S, D = 16, 2048, 64
NCORES = 8
BPC = B // NCORES  # batches per core
NT = S // 128  # 128-token tiles per batch
NTERMS = 12
F32 = mybir.dt.float32
F32R = mybir.dt.float32r
F16 = mybir.dt.float16
AF = mybir.ActivationFunctionType
AX = mybir.AxisListType
OP = mybir.AluOpType


def build_nc():
    nc = bacc.Bacc(None, target_bir_lowering=False)
    xin = nc.declare_dram_parameter("xin", [BPC, S, 66], F16, isOutput=False)
    wqk = nc.declare_dram_parameter("wqk", [128, 2 * D], F16, isOutput=False)
    w66 = nc.declare_dram_parameter("w66", [66, 66], F32R, isOutput=False)
    coef = nc.declare_dram_parameter("coef", [32, 1], F32, isOutput=False)
    out = nc.declare_dram_parameter("out", [BPC, S, D], F32, isOutput=True)

    with tile.TileContext(nc) as tc:
        with (
            tc.tile_pool(name="const", bufs=1) as constp,
            tc.tile_pool(name="xob", bufs=2) as xobp,
            tc.tile_pool(name="tmp", bufs=1) as tmpp,
            tc.tile_pool(name="small", bufs=2) as smallp,
            tc.tile_pool(name="fg", bufs=2) as fgp,
            tc.tile_pool(name="ft", bufs=2) as ftp,
            tc.tile_pool(name="ost", bufs=8) as ostp,
            tc.tile_pool(name="ps_m", bufs=2, space="PSUM") as ps_m,
            tc.tile_pool(name="ps_a", bufs=2, space="PSUM") as ps_a,
            tc.tile_pool(name="ps_po", bufs=4, space="PSUM") as ps_po,
        ):
            wqk_sb = constp.tile([128, 2 * D], F16)
            nc.sync.dma_start(wqk_sb[:], wqk[:])
            w66_sb = constp.tile([66, 66], F32R)
            coef_sb = constp.tile([32, 1], F32)

            # input arrives host-converted to fp16 with the ones columns
            # pre-appended ([x|1|1], 66 wide) — half the HBM traffic and no
            # on-device convert.  Batch 0 first on the scalar HWDGE.
            xobs = []
            for b in range(BPC):
                xob = xobp.tile([128, NT * 66], F16, tag="xob")
                xobs.append(xob)
            xv0 = xin[0].rearrange("(p a) d -> p a d", a=NT)
            xob0v = xobs[0][:].rearrange("p (a d) -> p a d", d=66)
            xv1 = xin[1].rearrange("(p a) d -> p a d", a=NT)
            xob1v = xobs[1][:].rearrange("p (a d) -> p a d", d=66)
            nc.scalar.dma_start(xob0v[:, 0:8, :], xv0[:, 0:8, :])
            nc.scalar.dma_start(xob0v[:, 8:16, :], xv0[:, 8:16, :])
            nc.scalar.dma_start(xob1v[:, 0:8, :], xv1[:, 0:8, :])
            nc.scalar.dma_start(xob1v[:, 8:16, :], xv1[:, 8:16, :])
            nc.sync.dma_start(w66_sb[:], w66[:])
            nc.sync.dma_start(coef_sb[:], coef[:])

            # PE warm-up: release the HAM clock gate while input DMAs fly.
            junk = smallp.tile([128, 256], F16, tag="junk")
            nc.gpsimd.memset(junk[:], 0.0)
            for _ in range(8):
                pjw = ps_po.tile([128, 264], F32, tag="po")
                nc.tensor.matmul(
                    pjw[:, 0:256], junk[:, 0:128], junk[:], start=True, stop=True
                )

            qks, fgs = [], []
            wqkv = wqk_sb[:].rearrange("p (o e d) -> p o e d", o=1, e=2)

            def prep(b):
                """q/k dot products + feature chain for batch b."""
                xobv = xobs[b][:].rearrange("p (a d) -> p a d", d=66)

                tA = tmpp.tile([128, NT * 2 * 64], F16, tag="tA")
                tAv = tA[:].rearrange("p (t e d) -> p t e d", t=NT, e=2)
                for h in range(2):
                    sl = slice(8 * h, 8 * h + 8)
                    xh = xobv[:, sl, 0:64].rearrange(
                        "p t (o d) -> p t o d", o=1
                    ).broadcast_to([128, 8, 2, D])
                    wh = wqkv.broadcast_to([128, 8, 2, D])
                    nc.vector.tensor_mul(tAv[:, sl, :, :], xh, wh)
                tB = tmpp.tile([128, NT * 2 * 32], F16, tag="tB")
                tBv = tB[:].rearrange("p (t e d) -> p t e d", t=NT, e=2)
                nc.vector.tensor_add(tBv, tAv[:, :, :, 0:32], tAv[:, :, :, 32:64])
                tC = tmpp.tile([128, NT * 2 * 16], F16, tag="tC")
                tCv = tC[:].rearrange("p (t e d) -> p t e d", t=NT, e=2)
                nc.vector.tensor_add(tCv, tBv[:, :, :, 0:16], tBv[:, :, :, 16:32])
                tD = tmpp.tile([128, NT * 2 * 8], F16, tag="tD")
                tDv = tD[:].rearrange("p (t e d) -> p t e d", t=NT, e=2)
                nc.vector.tensor_add(tDv, tCv[:, :, :, 0:8], tCv[:, :, :, 8:16])
                # fp16 q', k' straight out of the fold tree (4 summands of
                # bounded magnitude: fp16 rounding is input-perturbation-safe)
                qkh = smallp.tile([128, NT * 2], F16, tag="qkh")
                qkhv = qkh[:].rearrange("p (t e) -> p t e", e=2)
                with nc.allow_low_precision("fp16 qk fold output"):
                    nc.vector.tensor_reduce(qkhv, tDv, AX.X, OP.add)
                # feature chain in fp16: col (t, e, n): e=0 -> q'^n, e=1 -> k'^n w
                veng = nc.vector
                fg = fgp.tile([128, NT * 2 * 32], F16, tag="fg")
                fgs.append(fg)
                fgv = fg[:].rearrange("p (t e n) -> p t e n", e=2, n=32)
                nc.gpsimd.memset(fgv[:, :, 0:1, 0:1], 1.0)
                sqf = smallp.tile([128, NT], F32, tag="sqf")
                nc.scalar.activation(
                    sqf[:].rearrange("p (t o) -> p t o", o=1),
                    qkhv[:, :, 1:2],
                    AF.Square,
                )
                # w = exp(-2 k'^2) written directly into the g_0 feature slot
                nc.scalar.activation(
                    fgv[:, :, 1:2, 0:1],
                    sqf[:].rearrange("p (t e n) -> p t e n", e=1, n=1),
                    AF.Exp,
                    scale=-2.0,
                )
                qkh4 = qkhv.rearrange("p t (e o) -> p t e o", o=1)
                p2 = smallp.tile([128, NT * 2], F16, tag="p2")
                p2v = p2[:].rearrange("p (t e) -> p t e", e=2)
                veng.tensor_mul(p2v, qkhv, qkhv)
                p4 = smallp.tile([128, NT * 2], F16, tag="p4")
                p4v = p4[:].rearrange("p (t e) -> p t e", e=2)
                veng.tensor_mul(p4v, p2v, p2v)
                p4r = smallp.tile([128, NT * 2 * 4], F16, tag="p4r")
                p4rv = p4r[:].rearrange("p (t e n) -> p t e n", e=2, n=4)
                veng.tensor_copy(
                    p4rv,
                    p4v.rearrange("p t (e o) -> p t e o", o=1).broadcast_to(
                        [128, NT, 2, 4]
                    ),
                )
                p2b = p2v.rearrange("p t (e o) -> p t e o", o=1)
                # f-side chain first: depends only on qkh (no Act round-trip),
                # so the F^T transpose can launch before the g-side finishes
                veng.tensor_copy(fgv[:, :, 0:1, 1:2], qkh4[:, :, 0:1, :])
                veng.tensor_mul(
                    fgv[:, :, 0:1, 2:4],
                    fgv[:, :, 0:1, 0:2],
                    p2b[:, :, 0:1, :].broadcast_to([128, NT, 1, 2]),
                )
                veng.tensor_mul(
                    fgv[:, :, 0:1, 4:8], fgv[:, :, 0:1, 0:4], p4rv[:, :, 0:1, :]
                )
                veng.tensor_mul(
                    fgv[:, :, 0:1, 8:12], fgv[:, :, 0:1, 4:8], p4rv[:, :, 0:1, :]
                )
                # g-side chain (waits for w = exp(-2k'^2) in g_0)
                veng.tensor_mul(fgv[:, :, 1:2, 1:2], fgv[:, :, 1:2, 0:1], qkh4[:, :, 1:2, :])
                veng.tensor_mul(
                    fgv[:, :, 1:2, 2:4],
                    fgv[:, :, 1:2, 0:2],
                    p2b[:, :, 1:2, :].broadcast_to([128, NT, 1, 2]),
                )
                veng.tensor_mul(
                    fgv[:, :, 1:2, 4:8], fgv[:, :, 1:2, 0:4], p4rv[:, :, 1:2, :]
                )
                veng.tensor_mul(
                    fgv[:, :, 1:2, 8:12], fgv[:, :, 1:2, 4:8], p4rv[:, :, 1:2, :]
                )

            for b in range(BPC):
                prep(b)

            # per batch: G-matmuls + A fixup + coef scale + F^T + finals
            for b in range(BPC):
                xobv = xobs[b][:].rearrange("p (a d) -> p a d", d=66)
                fgv = fgs[b][:].rearrange("p (t e n) -> p t e n", e=2, n=32)
                pm = ps_m.tile([66, 32], F32, tag="m")
                for t in range(NT):
                    nc.tensor.matmul(
                        pm[:],
                        xobv[:, t, :],
                        fgv[:, t, 1, :],
                        start=(t == 0),
                        stop=(t == NT - 1),
                    )
                msb = smallp.tile([66, 32], F32R, tag="msb")
                nc.scalar.copy(msb[:], pm[:])
                pa = ps_a.tile([32, 66], F32, tag="a")
                nc.tensor.matmul(pa[:], msb[:], w66_sb[:], start=True, stop=True)
                atb = smallp.tile([32, 66], F16, tag="atb")
                nc.vector.tensor_scalar_mul(
                    atb[0:NTERMS, :], pa[0:NTERMS, :], coef_sb[0:NTERMS, :]
                )
                nc.gpsimd.memset(fgv[:, :, 0:1, 12:16], 0.0)
                if b == 0:
                    # batch 0: DVE stream-transpose (DVE frees up right after
                    # the chains; keeps the sync DGE clear for batch 1's XBAR)
                    ftb = ftp.tile([32, S], F16, tag="ftb32")
                    ftv = ftb[:].rearrange("p (t k i) -> p t k i", t=NT, k=4)
                    for kb in range(4):
                        nc.vector.transpose(
                            ftv[:, :, kb : kb + 1, :],
                            fgv[32 * kb : 32 * kb + 32, :, 0:1, :],
                        )
                else:
                    # batch 1: XBAR transpose — f staged into the low 16
                    # columns of 128-wide blocks; out[P,E,I] = ff[I, 128E+P],
                    # source columns >= the 16 out partitions are dropped.
                    ff = fgp.tile([128, 128 * NT], F16, tag="ff")
                    ffv = ff[:].rearrange("p (t c) -> p t c", c=128)
                    nc.scalar.copy(ffv[:, :, 0:16], fgv[:, :, 0, 0:16])
                    ftb = ftp.tile([16, S], F16, tag="ftb")
                    nc.sync.dma_start_transpose(
                        ftb[:].rearrange("p (t i) -> p t i", i=128), ff[:]
                    )

                # finals + normalize + output DMA for this batch
                ov = out[b].rearrange("(p a) d -> p a d", a=NT)
                for g in range(4):
                    po = ps_po.tile([128, 264], F32, tag="po")
                    pov = po[:].rearrange("p (k e) -> p k e", e=66)
                    for t2 in range(4):
                        t = 4 * g + t2
                        nc.tensor.matmul(
                            po[:, 66 * t2 : 66 * (t2 + 1)],
                            ftb[0:NTERMS, 128 * t : 128 * (t + 1)],
                            atb[0:NTERMS, :],
                            start=True,
                            stop=True,
                        )
                    rec = smallp.tile([128, 4], F32, tag="rec")
                    nc.vector.reciprocal(
                        rec[:].rearrange("p (k o) -> p k o", o=1), pov[:, :, 64:65]
                    )
                    ost = ostp.tile([128, 4 * 64], F32, tag="ost")
                    ostv = ost[:].rearrange("p (k d) -> p k d", k=4)
                    nc.vector.tensor_mul(
                        ostv,
                        pov[:, :, 0:64],
                        rec[:].rearrange("p (k o) -> p k o", o=1).broadcast_to(
                            [128, 4, 64]
                        ),
                    )
                    oeng = nc.sync if g % 2 == 0 else nc.scalar
                    oeng.dma_start(ov[:, 4 * g : 4 * g + 4, :], ostv)
    nc.compile()
    return nc


_NC_CACHE = None


def _get_nc():
    global _NC_CACHE
    if _NC_CACHE is None:
        _NC_CACHE = build_nc()
    return _NC_CACHE


def make_in_maps(input1, Wv, Wq, Wk):
    input1 = np.asarray(input1, dtype=np.float32)
    xin16 = np.empty((B, S, 66), np.float16)
    xin16[:, :, 0:64] = input1
    xin16[:, :, 64:66] = 1.0
    Wv = np.asarray(Wv, dtype=np.float32)
    Wq = np.asarray(Wq, dtype=np.float32)
    Wk = np.asarray(Wk, dtype=np.float32)
    wqk1 = np.stack([Wq / 4.0, Wk / 4.0]).reshape(1, 2 * D)
    wqk = np.broadcast_to(wqk1, (128, 2 * D)).astype(np.float16)
    w66 = np.zeros((66, 66), np.float32)
    w66[:64, :64] = Wv
    w66[64, 64] = 1.0
    coef = np.zeros((32, 1), np.float32)
    for n in range(NTERMS):
        coef[n] = 4.0**n / float(math.factorial(n))
    return [
        {
            "xin": np.ascontiguousarray(xin16[i * BPC : (i + 1) * BPC]),
            "wqk": np.ascontiguousarray(wqk),
            "w66": w66,
            "coef": coef,
        }
        for i in range(NCORES)
    ]


def kernel(input1, Wv, Wq, Wk):
    nc = _get_nc()
    in_maps = make_in_maps(input1, Wv, Wq, Wk)
    res = run_bass_kernel_spmd(nc, in_maps, core_ids=list(range(NCORES)))
    return np.concatenate([res.results[i]["out"] for i in range(NCORES)], axis=0)
